# revision 20
# baseline (speedup 1.0000x reference)
"""Multi-head latent attention (MLA) forward pass on 8 Trainium2 NeuronCores.

Sharding: 2 (batch) x 4 (head-group) grid. Core c handles batch b = c // 4
and heads 4*(c % 4) .. 4*(c % 4) + 3.

v3 design:
  - All layout transposes run on the DMA xbar engine (zero PE transposes);
    the sync HWDGE queue carries ONLY xbar transposes, while every bulk
    DRAM load/store goes through the gpsimd SWDGE queue (partition-major
    DRAM layouts keep descriptor counts at 1/partition), so transposes are
    never stuck behind megabyte loads.
  - s-loop: GEMM-A (q|krope|kv_down, 1280 cols) + norms + rope per s-tile;
    kv_up GEMM software-pipelined with a 2-tile lag so its ckv^T xbar
    transpose is long done.
  - attention per (q-block j, head h), k-major scores S^T[k,q] so the exp
    output IS the PV moving operand (no P transpose): per k-tile: score MM,
    exp on ACT, diagonal mask mult, DVE row-accumulate for the softmax
    denominator, PV MM (V stationary). The denominator finishes with a
    single ones^T @ acc matmul in float32r, reciprocal, and a gpsimd
    partition broadcast; the whole tail is emitted one (j,h) pair late so
    the PE queue never waits on the ACT/DVE chain.
  - out projection per q-block right after its 4 heads complete, stores on
    the SWDGE queue.
Host sums the 4 partials per batch element.
"""

import sys

for _p in ("/opt/trn_rl_repo",):
    if _p not in sys.path:
        sys.path.insert(0, _p)

import math
from contextlib import ExitStack

import ml_dtypes
import numpy as np

import concourse.bass as bass
import concourse.mybir as mybir
import concourse.tile as tile
from concourse import bacc
from concourse.bass_utils import run_bass_kernel_spmd

F32 = mybir.dt.float32
F32R = mybir.dt.float32r
BF16 = mybir.dt.bfloat16
BF = ml_dtypes.bfloat16

B, S, D = 2, 2048, 2048
H = 16
HD = 128           # head dim
ROPE = 64
NOPE = 64
LAT = 512
EPS = 1e-6
ROPE_BASE = 10000.0

H_LOC = 4          # heads per core
N_CORES = 8
DLOC = H_LOC * HD  # 512, per-core proj contraction size

ST_N = S // 128    # 16 s-tiles
KT_N = D // 128    # 16 k-tiles for GEMM-A
QB = 512           # attention q-block width

A_QW = H_LOC * HD             # 512  q columns in A
A_RW = H_LOC * ROPE           # 256  k_rope columns in A
LSL = LAT // 4                # 128  kv_down latent columns per core
A_W = A_QW + A_RW + LSL       # 896 total A columns
KV_W = H_LOC * NOPE + H_LOC * HD   # 768 kv columns

X8_CHUNK = 256                # s-columns of x^T per streamed chunk
X8_N = S // X8_CHUNK          # 8 chunks
ST_PER_CHUNK = X8_CHUNK // 128  # 2

MULT = mybir.AluOpType.mult
ADD = mybir.AluOpType.add
SUB = mybir.AluOpType.subtract
EXPF = mybir.ActivationFunctionType.Exp
SQRTF = mybir.ActivationFunctionType.Sqrt
SQF = mybir.ActivationFunctionType.Square
AXX = mybir.AxisListType.X
AXXY = mybir.AxisListType.XY

_PROGRAM_CACHE = {}


def _build_program():
    nc = bacc.Bacc(None, target_bir_lowering=False, debug=True)

    # ---- DRAM I/O (all partition-major so SWDGE descriptors stay 1/part) --
    xT8 = nc.dram_tensor("xT8", [X8_N, 128, KT_N, X8_CHUNK], BF16,
                         kind="ExternalInput")
    w_a = nc.dram_tensor("w_a", [128, KT_N, A_W], BF16, kind="ExternalInput")
    w_up = nc.dram_tensor("w_up", [128, LAT // 128, KV_W], BF16,
                          kind="ExternalInput")
    w_p = nc.dram_tensor("w_p", [128, H_LOC, D], BF16, kind="ExternalInput")
    cos4 = nc.dram_tensor("cos4", [128, ST_N, H_LOC, ROPE // 2], BF16,
                          kind="ExternalInput")
    sin4 = nc.dram_tensor("sin4", [128, ST_N, H_LOC, ROPE // 2], BF16,
                          kind="ExternalInput")
    masks = nc.dram_tensor("masks", [128, 4, QB], BF16, kind="ExternalInput")
    gain12 = nc.dram_tensor("gain12", [128, 12], F32, kind="ExternalInput")
    out = nc.dram_tensor("out", [S, D], F32, kind="ExternalOutput")
    stg = nc.dram_tensor("stg", [4, 4 * 128, LSL], BF16)       # raw ckv slice
    gath = nc.dram_tensor("gath", [4, 4, 4 * 128, LSL], BF16)  # gathered

    inv_sqrt_hd = 1.0 / math.sqrt(HD)

    with tile.TileContext(nc) as tc, ExitStack() as top:
        const = top.enter_context(tc.tile_pool(name="const", bufs=1))
        big = top.enter_context(tc.tile_pool(name="big", bufs=1))

        # --- persistent activations (head-dim-major) ---
        QT = big.tile([128, H_LOC, S], BF16)   # [d, h, q]
        KT = big.tile([128, H_LOC, S], BF16)   # [d, h, k] (0:64 nope, 64:128 rope)
        V = big.tile([128, ST_N, H_LOC * HD], BF16)  # [s%128, s//128, d_loc]
        yT = big.tile([128, H_LOC, S], BF16)   # [d, h, q]

        mask_sb = const.tile([128, 4, QB], BF16)
        gain_sb = const.tile([128, 12], F32)
        eps_sb = const.tile([128, 1], F32)
        ones_bf = const.tile([128, 1], BF16)
        wp_sb = const.tile([128, H_LOC, D], BF16)  # loaded late

        # ===================== phase S: s-tile loop =====================
        sphase = ExitStack()
        wpool = sphase.enter_context(tc.tile_pool(name="wpool", bufs=1))
        wa_sb = wpool.tile([128, KT_N, A_W], BF16)
        nc.scalar.dma_start(out=wa_sb[:, 0, :], in_=w_a[:, 0, :])

        x8p = sphase.enter_context(tc.tile_pool(name="x8p", bufs=2))
        xq0 = x8p.tile([128, KT_N, X8_CHUNK], BF16, tag="x8")
        nc.scalar.dma_start(out=xq0[:], in_=xT8[0])
        for kt in range(1, KT_N):
            nc.scalar.dma_start(out=wa_sb[:, kt, :], in_=w_a[:, kt, :])

        wup_sb = wpool.tile([128, LAT // 128, KV_W], BF16)
        nc.scalar.dma_start(out=wup_sb[:], in_=w_up[:])
        cos_sb = wpool.tile([128, ST_N, H_LOC, ROPE // 2], BF16)
        nc.scalar.dma_start(out=cos_sb[:], in_=cos4[:])
        sin_sb = wpool.tile([128, ST_N, H_LOC, ROPE // 2], BF16)
        nc.scalar.dma_start(out=sin_sb[:], in_=sin4[:])
        nc.scalar.dma_start(out=gain_sb[:], in_=gain12[:])
        nc.scalar.dma_start(out=mask_sb[:], in_=masks[:])
        nc.vector.memset(eps_sb[:], EPS)
        nc.vector.memset(ones_bf[:], 1.0)

        ckvT = big.tile([128, LAT // 128, S], BF16)  # [lat, lt, s]
        psA = sphase.enter_context(tc.tile_pool(name="psA", bufs=2, space="PSUM"))
        psKV = sphase.enter_context(tc.tile_pool(name="psKV", bufs=1, space="PSUM"))
        scr = sphase.enter_context(tc.tile_pool(name="scr", bufs=4))
        kcp = sphase.enter_context(tc.tile_pool(name="kcp", bufs=8))
        jnk = sphase.enter_context(tc.tile_pool(name="jnk", bufs=2))

        def rsqrt_act(dst, src, n):
            """dst = 1/sqrt(src/n + eps): ACT Sqrt then fast DVE reciprocal."""
            nc.scalar.activation(dst, src, SQRTF, scale=1.0 / n, bias=eps_sb[:])
            nc.vector.reciprocal_approx_fast(out=dst, in_=dst)

        xq1 = x8p.tile([128, KT_N, X8_CHUNK], BF16, tag="x8")
        nc.scalar.dma_start(out=xq1[:], in_=xT8[1])
        xq_next = {0: xq0, 1: xq1}

        def emit_gemm_a(ST):
            e, st2 = divmod(ST, ST_PER_CHUNK)
            if st2 == 0 and e + 1 < X8_N and (e + 1) not in xq_next:
                nxt = x8p.tile([128, KT_N, X8_CHUNK], BF16, tag="x8")
                nc.scalar.dma_start(out=nxt[:], in_=xT8[e + 1])
                xq_next[e + 1] = nxt
            xq = xq_next[e]
            aps = psA.tile([128, A_W], F32, tag="A")
            for kt in range(KT_N):
                lhs = xq[:, kt, st2 * 128:(st2 + 1) * 128]
                for c0, c1 in ((0, 512), (512, 896)):
                    nc.tensor.matmul(
                        aps[:, c0:c1], lhs, wa_sb[:, kt, c0:c1],
                        start=(kt == 0), stop=(kt == KT_N - 1))
            return aps

        def emit_norms(ST, aps):
            s0 = ST * 128
            junk = jnk.tile([128, 768], BF16, tag="junk")
            nc.scalar.activation(junk[:], aps[:, 0:768], SQF)
            rs13 = scr.tile([128, 12], F32, tag="rs13")
            nc.vector.tensor_reduce(
                rs13[:],
                junk[:].rearrange("p (g c) -> p g c", c=64),
                AXX, ADD)
            rsqrt_act(rs13[:], rs13[:], 64)
            nc.vector.tensor_tensor(rs13[:], rs13[:], gain_sb[:], MULT)

            # ---- apply norms ----
            nrmq = scr.tile([128, A_QW], BF16, tag="nrmq")
            nc.vector.tensor_tensor(
                nrmq[:].rearrange("p (g c) -> p g c", c=64),
                aps[:, 0:512].rearrange("p (g c) -> p g c", c=64),
                rs13[:, 0:8].to_broadcast([128, 8, 64]), MULT)
            kcomb = kcp.tile([128, H_LOC, HD], BF16, tag="kcomb")
            nc.vector.tensor_tensor(
                kcomb[:, :, NOPE:HD],
                aps[:, 512:768].rearrange("p (h c) -> p h c", c=64),
                rs13[:, 8:12].to_broadcast([128, 4, 64]), MULT)
            cs = scr.tile([128, LSL], BF16, tag="cs")
            nc.scalar.copy(cs[:], aps[:, 768:896])
            nc.gpsimd.dma_start(
                out=stg[ST // 4, (ST % 4) * 128:(ST % 4 + 1) * 128, :],
                in_=cs[:])

            # ---- rope (in place; temps carry all products first) ----
            RH = ROPE // 2
            cos_ap = cos_sb[:, ST]
            sin_ap = sin_sb[:, ST]
            nq = nrmq[:].rearrange("p (h t c) -> p h t c", t=2, c=64)

            def rope_inplace(x1, x2):
                t1 = scr.tile([128, H_LOC, RH], F32, tag="t1")
                t2 = scr.tile([128, H_LOC, RH], F32, tag="t2")
                t3 = scr.tile([128, H_LOC, RH], F32, tag="t3")
                t4 = scr.tile([128, H_LOC, RH], F32, tag="t4")
                nc.vector.tensor_tensor(t1[:], x1, cos_ap, MULT)
                nc.vector.tensor_tensor(t2[:], x2, sin_ap, MULT)
                nc.vector.tensor_tensor(t3[:], x2, cos_ap, MULT)
                nc.vector.tensor_tensor(t4[:], x1, sin_ap, MULT)
                nc.vector.tensor_tensor(x1, t1[:], t2[:], ADD)
                nc.vector.tensor_tensor(x2, t3[:], t4[:], SUB)

            rope_inplace(nq[:, :, 1, 0:RH], nq[:, :, 1, RH:ROPE])
            rope_inplace(kcomb[:, :, NOPE:NOPE + RH], kcomb[:, :, NOPE + RH:HD])

            # ---- DMA xbar transposes into head-dim-major tiles ----
            nc.sync.dma_start(
                out=QT[:, :, s0:s0 + 128], in_=nrmq[:], transpose=True)
            return kcomb

        rs_blk = {}  # block -> [128, 4] f32 V-scale tile

        def emit_gather(b):
            """AllGather block b's raw ckv slices, rebuild ckvT + V-scale."""
            groups = [[0, 1, 2, 3], [4, 5, 6, 7]]
            nc.gpsimd.collective_compute(
                "AllGather", mybir.AluOpType.bypass, groups,
                ins=[stg[b]], outs=[gath[b]])
            q0 = b * 512
            for lt in range(4):
                nc.sync.dma_start(
                    out=ckvT[:, lt, q0:q0 + 512], in_=gath[b, lt],
                    transpose=True)
            csg = scr.tile([128, 4, 4, LSL], BF16, tag="csg")
            for lt in range(4):
                nc.scalar.dma_start(
                    out=csg[:, :, lt, :],
                    in_=gath[b, lt].rearrange("(t p) c -> p t c", p=128))
            jg = jnk.tile([128, 4, 4 * LSL], BF16, tag="jg")
            nc.scalar.activation(
                jg[:], csg[:].rearrange("p t l c -> p t (l c)"), SQF)
            rsb = scr.tile([128, 4], F32, tag="rsb")
            nc.vector.tensor_reduce(rsb[:], jg[:], AXX, ADD)
            rsqrt_act(rsb[:], rsb[:], LAT)
            rs_blk[b] = rsb

        def emit_kv_up(ST, kcomb):
            s0 = ST * 128
            kvps = psKV.tile([128, KV_W], F32, tag="KV")
            for lt in range(LAT // 128):
                lhs = ckvT[:, lt, s0:s0 + 128]
                for c0, c1 in ((0, 512), (512, 768)):
                    nc.tensor.matmul(
                        kvps[:, c0:c1], lhs, wup_sb[:, lt, c0:c1],
                        start=(lt == 0), stop=(lt == LAT // 128 - 1))
            junkk = jnk.tile([128, H_LOC * NOPE], BF16, tag="junkk")
            nc.scalar.activation(junkk[:], kvps[:, 0:256], SQF)
            rsk = scr.tile([128, 4], F32, tag="rsk")
            nc.vector.tensor_reduce(
                rsk[:], junkk[:].rearrange("p (g c) -> p g c", c=64),
                AXX, ADD)
            rsqrt_act(rsk[:], rsk[:], 64)
            nc.vector.tensor_tensor(
                kcomb[:, :, 0:NOPE],
                kvps[:, 0:256].rearrange("p (g c) -> p g c", c=64),
                rsk[:].to_broadcast([128, 4, 64]), MULT)
            nc.vector.tensor_scalar(
                V[:, ST, :], kvps[:, H_LOC * NOPE:KV_W],
                rs_blk[ST // 4][:, ST % 4:ST % 4 + 1], None, MULT)
            nc.sync.dma_start(
                out=KT[:, :, s0:s0 + 128], in_=kcomb[:], transpose=True)

        pend = []  # (ST, kcomb) with norms emitted, kv_up not yet (lag 6)
        for ST in range(ST_N):
            aps = emit_gemm_a(ST)
            if len(pend) >= 6:
                emit_kv_up(*pend.pop(0))
            kcomb = emit_norms(ST, aps)
            pend.append((ST, kcomb))
            if ST % 4 == 3:
                emit_gather(ST // 4)
        for p in pend:
            emit_kv_up(*p)

        sphase.close()

        # late load for attention/proj phase
        nc.scalar.dma_start(out=wp_sb[:], in_=w_p[:])

        # =========== phase T: attention + out projection per q-block ========
        with (
            tc.tile_pool(name="pS", bufs=2, space="PSUM") as pS,
            tc.tile_pool(name="pY", bufs=2, space="PSUM") as pY,
            tc.tile_pool(name="pL", bufs=2, space="PSUM") as pL,
            tc.tile_pool(name="pO", bufs=2, space="PSUM") as pO,
            tc.tile_pool(name="pp", bufs=8) as pp,
            tc.tile_pool(name="rp", bufs=4) as rp,
            tc.tile_pool(name="po", bufs=4) as po,
        ):
            def emit_proj_group(t, nb):
                """One out-projection PSUM tile: 4 head matmuls + store."""
                s0 = t * 128
                ot = pO.tile([128, 512], F32, tag="O")
                for h in range(H_LOC):
                    nc.tensor.matmul(
                        ot[:], yT[:, h, s0:s0 + 128],
                        wp_sb[:, h, nb * 512:(nb + 1) * 512],
                        start=(h == 0), stop=(h == H_LOC - 1))
                osb = po.tile([128, 512], F32, tag="osb")
                nc.vector.tensor_copy(osb[:], ot[:])
                nc.gpsimd.dma_start(
                    out=out[s0:s0 + 128, nb * 512:(nb + 1) * 512],
                    in_=osb[:])

            def emit_pair(j, h, fillers):
                """Scores + PV + denominator for q-block j, head h. PV and
                the ones-matmul row-sum trail the scores by 2 k-tiles so
                the exp chain hides under matmul streams; `fillers` are
                pending proj groups emitted mid-pair as extra PE work."""
                q0 = j * QB
                nkt = 4 * j + 4
                lps = pL.tile([1, QB], F32, tag="L")
                yps = pY.tile([128, QB], F32, tag="Y")
                Ps = []
                for kt in range(nkt + 2):
                    if kt < nkt:
                        sps = pS.tile([128, QB], F32, tag="S")
                        nc.tensor.matmul(
                            sps[:], KT[:, h, kt * 128:(kt + 1) * 128],
                            QT[:, h, q0:q0 + QB], start=True, stop=True)
                        P = pp.tile([128, QB], BF16, tag="P")
                        nc.scalar.activation(
                            P[:], sps[:], EXPF, scale=inv_sqrt_hd)
                        d_idx = kt - 4 * j
                        if d_idx >= 0:
                            nc.vector.tensor_tensor(
                                P[:], P[:], mask_sb[:, d_idx, :], MULT)
                        Ps.append(P)
                    pv = kt - 2
                    if pv >= 0:
                        nc.tensor.matmul(
                            yps[:], V[:, pv, h * HD:(h + 1) * HD], Ps[pv][:],
                            start=(pv == 0), stop=(pv == nkt - 1))
                        nc.tensor.matmul(
                            lps[:], ones_bf[:], Ps[pv][:],
                            start=(pv == 0), stop=(pv == nkt - 1))
                    if kt % 4 == 1 and fillers:
                        emit_proj_group(*fillers.pop(0))
                return (j, h, lps, yps)

            def emit_tail(j, h, lps, yps):
                """1/l partition-broadcast and y scaling into yT."""
                q0 = j * QB
                rinv = rp.tile([1, QB], F32, tag="rinv")
                nc.vector.reciprocal_approx_fast(out=rinv[:], in_=lps[:])
                rbc = rp.tile([128, QB], F32, tag="rbc")
                nc.gpsimd.partition_broadcast(rbc[:], rinv[:])
                nc.vector.tensor_tensor(
                    yT[:, h, q0:q0 + QB], yps[:], rbc[:], MULT)

            pend_tail = None
            proj_fifo = []   # (t, nb) proj groups awaiting emission
            for j in range(S // QB):
                for h in range(H_LOC):
                    state = emit_pair(j, h, proj_fifo)
                    if pend_tail is not None:
                        emit_tail(*pend_tail)
                        if pend_tail[1] == H_LOC - 1:
                            jj = pend_tail[0]
                            proj_fifo.extend(
                                (t, nb) for t in range(4 * jj, 4 * jj + 4)
                                for nb in range(D // 512))
                    pend_tail = state
            emit_tail(*pend_tail)
            jj = pend_tail[0]
            proj_fifo.extend(
                (t, nb) for t in range(4 * jj, 4 * jj + 4)
                for nb in range(D // 512))
            while proj_fifo:
                emit_proj_group(*proj_fifo.pop(0))
    nc.compile()
    return nc


def _prep_inputs(x, w_q_krope, w_kv_down, w_kv_up, w_proj, q_gain):
    """Build the 8 per-core input maps (host-side sharding, partition-major)."""

    def pmaj(a, p=128):
        """[K*p, ...] row-major -> [p, K, ...] partition-major contiguous."""
        k = a.shape[0] // p
        return np.ascontiguousarray(
            a.reshape((k, p) + a.shape[1:]).swapaxes(0, 1))

    inv_freq = ROPE_BASE ** (-np.arange(0, ROPE, 2, dtype=np.float32) / ROPE)
    t = np.arange(S, dtype=np.float32)
    freqs = np.outer(t, inv_freq)                      # (S, 32)
    cos4 = pmaj(np.ascontiguousarray(np.broadcast_to(
        np.cos(freqs)[:, None, :], (S, H_LOC, ROPE // 2))).astype(BF))
    sin4 = pmaj(np.ascontiguousarray(np.broadcast_to(
        np.sin(freqs)[:, None, :], (S, H_LOC, ROPE // 2))).astype(BF))

    kk = np.arange(128)[:, None, None]
    dd = np.arange(4)[None, :, None]
    qq = np.arange(QB)[None, None, :]
    masks = (kk + 128 * dd <= qq).astype(BF)           # [128, 4, QB]

    # x^T per batch, chunked partition-major: [X8_N, 128, KT_N, X8_CHUNK]
    xT_chunks = []
    for b in range(B):
        xT = np.ascontiguousarray(x[b].T).astype(BF)   # [D, S]
        ch = xT.reshape(KT_N, 128, X8_N, X8_CHUNK)
        xT_chunks.append(np.ascontiguousarray(ch.transpose(2, 1, 0, 3)))

    in_maps = []
    for c in range(N_CORES):
        b = c // H_LOC
        hg = c % H_LOC
        heads = [hg * H_LOC + i for i in range(H_LOC)]
        w_a = np.concatenate(
            [w_q_krope[:, h * HD:(h + 1) * HD] for h in heads]
            + [w_q_krope[:, D + h * ROPE:D + (h + 1) * ROPE] for h in heads]
            + [w_kv_down[:, hg * LSL:(hg + 1) * LSL]], axis=1).astype(BF)
        w_up = np.concatenate(
            [w_kv_up[:, h * NOPE:(h + 1) * NOPE] for h in heads]
            + [w_kv_up[:, NOPE * H + h * HD:NOPE * H + (h + 1) * HD]
               for h in heads], axis=1).astype(BF)      # [LAT, 768]
        w_p = w_proj[hg * DLOC:(hg + 1) * DLOC, :].astype(BF)   # [512, D]
        g = q_gain[heads].astype(np.float32)
        g12 = np.concatenate([np.repeat(g, 2), np.ones(4, np.float32)])
        gain12 = np.ascontiguousarray(
            np.broadcast_to(g12[None, :], (128, 12))).astype(np.float32)
        in_maps.append({
            "xT8": xT_chunks[b],
            "w_a": pmaj(w_a),
            "w_up": pmaj(w_up),
            "w_p": pmaj(w_p),
            "cos4": cos4, "sin4": sin4, "masks": masks,
            "gain12": gain12,
        })
    return in_maps


def kernel(x, w_q_krope, w_kv_down, w_kv_up, w_proj, q_gain, **_unused):
    x = np.asarray(x, dtype=np.float32)
    w_q_krope = np.asarray(w_q_krope, dtype=np.float32)
    w_kv_down = np.asarray(w_kv_down, dtype=np.float32)
    w_kv_up = np.asarray(w_kv_up, dtype=np.float32)
    w_proj = np.asarray(w_proj, dtype=np.float32)
    q_gain = np.asarray(q_gain, dtype=np.float32)

    if "nc" not in _PROGRAM_CACHE:
        _PROGRAM_CACHE["nc"] = _build_program()
    nc = _PROGRAM_CACHE["nc"]

    in_maps = _prep_inputs(x, w_q_krope, w_kv_down, w_kv_up, w_proj, q_gain)
    res = run_bass_kernel_spmd(nc, in_maps, list(range(N_CORES)))

    out = np.zeros((B, S, D), dtype=np.float32)
    for c in range(N_CORES):
        out[c // H_LOC] += res.results[c]["out"]
    return out


# revision 22
# speedup vs baseline: 1.3763x; 1.3763x over previous
"""Multi-head latent attention (MLA) forward pass on 8 Trainium2 NeuronCores.

Sharding: 2 (batch) x 4 (head-group) grid. Core c handles batch b = c // 4
and heads 4*(c % 4) .. 4*(c % 4) + 3.

v3 design:
  - All layout transposes run on the DMA xbar engine (zero PE transposes);
    the sync HWDGE queue carries ONLY xbar transposes, while every bulk
    DRAM load/store goes through the gpsimd SWDGE queue (partition-major
    DRAM layouts keep descriptor counts at 1/partition), so transposes are
    never stuck behind megabyte loads.
  - s-loop: GEMM-A (q|krope|kv_down, 1280 cols) + norms + rope per s-tile;
    kv_up GEMM software-pipelined with a 2-tile lag so its ckv^T xbar
    transpose is long done.
  - attention per (q-block j, head h), k-major scores S^T[k,q] so the exp
    output IS the PV moving operand (no P transpose): per k-tile: score MM,
    exp on ACT, diagonal mask mult, DVE row-accumulate for the softmax
    denominator, PV MM (V stationary). The denominator finishes with a
    single ones^T @ acc matmul in float32r, reciprocal, and a gpsimd
    partition broadcast; the whole tail is emitted one (j,h) pair late so
    the PE queue never waits on the ACT/DVE chain.
  - out projection per q-block right after its 4 heads complete, stores on
    the SWDGE queue.
Host sums the 4 partials per batch element.
"""

import sys

for _p in ("/opt/trn_rl_repo",):
    if _p not in sys.path:
        sys.path.insert(0, _p)

import math
from contextlib import ExitStack

import ml_dtypes
import numpy as np

import concourse.bass as bass
import concourse.mybir as mybir
import concourse.tile as tile
from concourse import bacc
from concourse.bass_utils import run_bass_kernel_spmd

F32 = mybir.dt.float32
F32R = mybir.dt.float32r
BF16 = mybir.dt.bfloat16
BF = ml_dtypes.bfloat16

B, S, D = 2, 2048, 2048
H = 16
HD = 128           # head dim
ROPE = 64
NOPE = 64
LAT = 512
EPS = 1e-6
ROPE_BASE = 10000.0

H_LOC = 4          # heads per core
N_CORES = 8
DLOC = H_LOC * HD  # 512, per-core proj contraction size

ST_N = S // 128    # 16 s-tiles
KT_N = D // 128    # 16 k-tiles for GEMM-A
QB = 512           # attention q-block width

A_QW = H_LOC * HD             # 512  q columns in A
A_RW = H_LOC * ROPE           # 256  k_rope columns in A
A_W = A_QW + A_RW + LAT       # 1280 total A columns
KV_W = H_LOC * NOPE + H_LOC * HD   # 768 kv columns

X8_CHUNK = 256                # s-columns of x^T per streamed chunk
X8_N = S // X8_CHUNK          # 8 chunks
ST_PER_CHUNK = X8_CHUNK // 128  # 2

MULT = mybir.AluOpType.mult
ADD = mybir.AluOpType.add
SUB = mybir.AluOpType.subtract
EXPF = mybir.ActivationFunctionType.Exp
SQRTF = mybir.ActivationFunctionType.Sqrt
SQF = mybir.ActivationFunctionType.Square
AXX = mybir.AxisListType.X
AXXY = mybir.AxisListType.XY

_PROGRAM_CACHE = {}


def _build_program():
    nc = bacc.Bacc(None, target_bir_lowering=False, debug=True)

    # ---- DRAM I/O (all partition-major so SWDGE descriptors stay 1/part) --
    xT8 = nc.dram_tensor("xT8", [X8_N, 128, KT_N, X8_CHUNK], BF16,
                         kind="ExternalInput")
    w_a = nc.dram_tensor("w_a", [128, KT_N, A_W], BF16, kind="ExternalInput")
    w_up = nc.dram_tensor("w_up", [128, LAT // 128, KV_W], BF16,
                          kind="ExternalInput")
    w_p = nc.dram_tensor("w_p", [128, H_LOC, D], BF16, kind="ExternalInput")
    cos4 = nc.dram_tensor("cos4", [128, ST_N, H_LOC, ROPE // 2], BF16,
                          kind="ExternalInput")
    sin4 = nc.dram_tensor("sin4", [128, ST_N, H_LOC, ROPE // 2], BF16,
                          kind="ExternalInput")
    masks = nc.dram_tensor("masks", [128, 4, QB], BF16, kind="ExternalInput")
    gain13 = nc.dram_tensor("gain13", [128, 13], F32, kind="ExternalInput")
    out = nc.dram_tensor("out", [S, D], F32, kind="ExternalOutput")

    inv_sqrt_hd = 1.0 / math.sqrt(HD)

    with tile.TileContext(nc) as tc, ExitStack() as top:
        const = top.enter_context(tc.tile_pool(name="const", bufs=1))
        big = top.enter_context(tc.tile_pool(name="big", bufs=1))

        # --- persistent activations (head-dim-major) ---
        QT = big.tile([128, H_LOC, S], BF16)   # [d, h, q]
        KT = big.tile([128, H_LOC, S], BF16)   # [d, h, k] (0:64 nope, 64:128 rope)
        V = big.tile([128, ST_N, H_LOC * HD], BF16)  # [s%128, s//128, d_loc]
        yT = big.tile([128, H_LOC, S], BF16)   # [d, h, q]

        mask_sb = const.tile([128, 4, QB], BF16)
        gain_sb = const.tile([128, 13], F32)
        eps_sb = const.tile([128, 1], F32)
        ones_bf = const.tile([128, 1], BF16)
        wp_sb = const.tile([128, H_LOC, D], BF16)  # loaded late

        # ===================== phase S: s-tile loop =====================
        sphase = ExitStack()
        wpool = sphase.enter_context(tc.tile_pool(name="wpool", bufs=1))
        wa_sb = wpool.tile([128, KT_N, A_W], BF16)
        nc.scalar.dma_start(out=wa_sb[:, 0, :], in_=w_a[:, 0, :])

        x8p = sphase.enter_context(tc.tile_pool(name="x8p", bufs=2))
        xq0 = x8p.tile([128, KT_N, X8_CHUNK], BF16, tag="x8")
        nc.scalar.dma_start(out=xq0[:], in_=xT8[0])
        for kt in range(1, KT_N):
            nc.scalar.dma_start(out=wa_sb[:, kt, :], in_=w_a[:, kt, :])

        wup_sb = wpool.tile([128, LAT // 128, KV_W], BF16)
        nc.scalar.dma_start(out=wup_sb[:], in_=w_up[:])
        cos_sb = wpool.tile([128, ST_N, H_LOC, ROPE // 2], BF16)
        nc.scalar.dma_start(out=cos_sb[:], in_=cos4[:])
        sin_sb = wpool.tile([128, ST_N, H_LOC, ROPE // 2], BF16)
        nc.scalar.dma_start(out=sin_sb[:], in_=sin4[:])
        nc.scalar.dma_start(out=gain_sb[:], in_=gain13[:])
        nc.scalar.dma_start(out=mask_sb[:], in_=masks[:])
        nc.vector.memset(eps_sb[:], EPS)
        nc.vector.memset(ones_bf[:], 1.0)

        # warm the PE clock (HAM) and ACT tables while the preamble loads:
        # dummy matmuls on a zeroed tile + one Square/Sqrt/Exp round.
        warm = wpool.tile([128, 512], BF16)
        nc.vector.memset(warm[:], 0.0)
        wps = None
        with tc.tile_pool(name="pw", bufs=1, space="PSUM") as pw:
            wps = pw.tile([128, 512], F32)
            for i in range(24):
                nc.tensor.matmul(wps[:], warm[:, 0:128], warm[:],
                                 start=True, stop=True)
            wsb = wpool.tile([128, 16], F32)
            nc.scalar.activation(wsb[:], wps[:, 0:16], SQF)
            nc.scalar.activation(wsb[:], wps[:, 0:16], SQRTF,
                                 bias=eps_sb[:])
            nc.scalar.activation(wsb[:], wps[:, 0:16], EXPF)

        ckvT = big.tile([128, LAT // 128, S], BF16)  # [lat, lt, s]
        psA = sphase.enter_context(tc.tile_pool(name="psA", bufs=2, space="PSUM"))
        psKV = sphase.enter_context(tc.tile_pool(name="psKV", bufs=1, space="PSUM"))
        scr = sphase.enter_context(tc.tile_pool(name="scr", bufs=4))
        jnk = sphase.enter_context(tc.tile_pool(name="jnk", bufs=2))

        def rsqrt_act(dst, src, n):
            """dst = 1/sqrt(src/n + eps): ACT Sqrt then fast DVE reciprocal."""
            nc.scalar.activation(dst, src, SQRTF, scale=1.0 / n, bias=eps_sb[:])
            nc.vector.reciprocal_approx_fast(out=dst, in_=dst)

        xq1 = x8p.tile([128, KT_N, X8_CHUNK], BF16, tag="x8")
        nc.scalar.dma_start(out=xq1[:], in_=xT8[1])
        xq_next = {0: xq0, 1: xq1}

        def emit_gemm_a(ST):
            e, st2 = divmod(ST, ST_PER_CHUNK)
            if st2 == 0 and e + 1 < X8_N and (e + 1) not in xq_next:
                nxt = x8p.tile([128, KT_N, X8_CHUNK], BF16, tag="x8")
                nc.scalar.dma_start(out=nxt[:], in_=xT8[e + 1])
                xq_next[e + 1] = nxt
            xq = xq_next[e]
            aps = psA.tile([128, A_W], F32, tag="A")
            for kt in range(KT_N):
                lhs = xq[:, kt, st2 * 128:(st2 + 1) * 128]
                for c0, c1 in ((0, 512), (512, 1024), (1024, 1280)):
                    nc.tensor.matmul(
                        aps[:, c0:c1], lhs, wa_sb[:, kt, c0:c1],
                        start=(kt == 0), stop=(kt == KT_N - 1))
            return aps

        def emit_norms(ST, aps):
            s0 = ST * 128
            junk = jnk.tile([128, A_W], BF16, tag="junk")
            nc.scalar.activation(junk[:], aps[:], SQF)
            rs13 = scr.tile([128, 13], F32, tag="rs13")
            nc.vector.tensor_reduce(
                rs13[:, 0:12],
                junk[:, 0:768].rearrange("p (g c) -> p g c", c=64),
                AXX, ADD)
            nc.vector.tensor_reduce(
                rs13[:, 12:13],
                junk[:, 768:1280].rearrange("p (g c) -> p g c", c=64),
                AXXY, ADD)
            rsqrt_act(rs13[:, 0:12], rs13[:, 0:12], 64)
            rsqrt_act(rs13[:, 12:13], rs13[:, 12:13], LAT)
            nc.vector.tensor_tensor(rs13[:], rs13[:], gain_sb[:], MULT)

            # ---- apply norms ----
            nrmq = scr.tile([128, A_QW], BF16, tag="nrmq")
            nc.vector.tensor_tensor(
                nrmq[:].rearrange("p (g c) -> p g c", c=64),
                aps[:, 0:512].rearrange("p (g c) -> p g c", c=64),
                rs13[:, 0:8].to_broadcast([128, 8, 64]), MULT)
            kcomb = scr.tile([128, H_LOC, HD], BF16, tag="kcomb")
            nc.vector.tensor_tensor(
                kcomb[:, :, NOPE:HD],
                aps[:, 512:768].rearrange("p (h c) -> p h c", c=64),
                rs13[:, 8:12].to_broadcast([128, 4, 64]), MULT)
            cv = scr.tile([128, LAT], BF16, tag="cv")
            nc.vector.tensor_scalar(
                cv[:], aps[:, 768:1280], rs13[:, 12:13], None, MULT)

            # ---- rope (in place; temps carry all products first) ----
            RH = ROPE // 2
            cos_ap = cos_sb[:, ST]
            sin_ap = sin_sb[:, ST]
            nq = nrmq[:].rearrange("p (h t c) -> p h t c", t=2, c=64)

            def rope_inplace(x1, x2):
                t1 = scr.tile([128, H_LOC, RH], F32, tag="t1")
                t2 = scr.tile([128, H_LOC, RH], F32, tag="t2")
                t3 = scr.tile([128, H_LOC, RH], F32, tag="t3")
                t4 = scr.tile([128, H_LOC, RH], F32, tag="t4")
                nc.vector.tensor_tensor(t1[:], x1, cos_ap, MULT)
                nc.vector.tensor_tensor(t2[:], x2, sin_ap, MULT)
                nc.vector.tensor_tensor(t3[:], x2, cos_ap, MULT)
                nc.vector.tensor_tensor(t4[:], x1, sin_ap, MULT)
                nc.vector.tensor_tensor(x1, t1[:], t2[:], ADD)
                nc.vector.tensor_tensor(x2, t3[:], t4[:], SUB)

            rope_inplace(nq[:, :, 1, 0:RH], nq[:, :, 1, RH:ROPE])
            rope_inplace(kcomb[:, :, NOPE:NOPE + RH], kcomb[:, :, NOPE + RH:HD])

            # ---- DMA xbar transposes into head-dim-major tiles ----
            nc.sync.dma_start(
                out=QT[:, :, s0:s0 + 128], in_=nrmq[:], transpose=True)
            nc.sync.dma_start(
                out=ckvT[:, :, s0:s0 + 128], in_=cv[:], transpose=True)
            return kcomb

        def emit_kv_up(ST, kcomb):
            s0 = ST * 128
            kvps = psKV.tile([128, KV_W], F32, tag="KV")
            for lt in range(LAT // 128):
                lhs = ckvT[:, lt, s0:s0 + 128]
                for c0, c1 in ((0, 512), (512, 768)):
                    nc.tensor.matmul(
                        kvps[:, c0:c1], lhs, wup_sb[:, lt, c0:c1],
                        start=(lt == 0), stop=(lt == LAT // 128 - 1))
            junkk = jnk.tile([128, H_LOC * NOPE], BF16, tag="junkk")
            nc.scalar.activation(junkk[:], kvps[:, 0:256], SQF)
            rsk = scr.tile([128, 4], F32, tag="rsk")
            nc.vector.tensor_reduce(
                rsk[:], junkk[:].rearrange("p (g c) -> p g c", c=64),
                AXX, ADD)
            rsqrt_act(rsk[:], rsk[:], 64)
            nc.vector.tensor_tensor(
                kcomb[:, :, 0:NOPE],
                kvps[:, 0:256].rearrange("p (g c) -> p g c", c=64),
                rsk[:].to_broadcast([128, 4, 64]), MULT)
            nc.scalar.copy(V[:, ST, :], kvps[:, H_LOC * NOPE:KV_W])
            nc.sync.dma_start(
                out=KT[:, :, s0:s0 + 128], in_=kcomb[:], transpose=True)

        pend = []  # (ST, kcomb) with norms emitted, kv_up not yet (lag 2)
        for ST in range(ST_N):
            aps = emit_gemm_a(ST)
            if len(pend) >= 2:
                emit_kv_up(*pend.pop(0))
            kcomb = emit_norms(ST, aps)
            pend.append((ST, kcomb))
        for p in pend:
            emit_kv_up(*p)

        sphase.close()

        # late load for attention/proj phase
        nc.scalar.dma_start(out=wp_sb[:], in_=w_p[:])

        # =========== phase T: attention + out projection per q-block ========
        with (
            tc.tile_pool(name="pS", bufs=2, space="PSUM") as pS,
            tc.tile_pool(name="pY", bufs=2, space="PSUM") as pY,
            tc.tile_pool(name="pL", bufs=2, space="PSUM") as pL,
            tc.tile_pool(name="pO", bufs=2, space="PSUM") as pO,
            tc.tile_pool(name="pp", bufs=8) as pp,
            tc.tile_pool(name="rp", bufs=4) as rp,
            tc.tile_pool(name="po", bufs=4) as po,
        ):
            def emit_proj_group(t, nb):
                """One out-projection PSUM tile: 4 head matmuls + store."""
                s0 = t * 128
                ot = pO.tile([128, 512], F32, tag="O")
                for h in range(H_LOC):
                    nc.tensor.matmul(
                        ot[:], yT[:, h, s0:s0 + 128],
                        wp_sb[:, h, nb * 512:(nb + 1) * 512],
                        start=(h == 0), stop=(h == H_LOC - 1))
                osb = po.tile([128, 512], F32, tag="osb")
                nc.vector.tensor_copy(osb[:], ot[:])
                nc.gpsimd.dma_start(
                    out=out[s0:s0 + 128, nb * 512:(nb + 1) * 512],
                    in_=osb[:])

            def emit_pair(j, h, fillers):
                """Scores + PV + denominator for q-block j, head h. PV and
                the ones-matmul row-sum trail the scores by 2 k-tiles so
                the exp chain hides under matmul streams; `fillers` are
                pending proj groups emitted mid-pair as extra PE work."""
                q0 = j * QB
                nkt = 4 * j + 4
                lps = pL.tile([1, QB], F32, tag="L")
                yps = pY.tile([128, QB], F32, tag="Y")
                Ps = []
                for kt in range(nkt + 2):
                    if kt < nkt:
                        sps = pS.tile([128, QB], F32, tag="S")
                        nc.tensor.matmul(
                            sps[:], KT[:, h, kt * 128:(kt + 1) * 128],
                            QT[:, h, q0:q0 + QB], start=True, stop=True)
                        P = pp.tile([128, QB], BF16, tag="P")
                        nc.scalar.activation(
                            P[:], sps[:], EXPF, scale=inv_sqrt_hd)
                        d_idx = kt - 4 * j
                        if d_idx >= 0:
                            nc.vector.tensor_tensor(
                                P[:], P[:], mask_sb[:, d_idx, :], MULT)
                        Ps.append(P)
                    pv = kt - 2
                    if pv >= 0:
                        nc.tensor.matmul(
                            yps[:], V[:, pv, h * HD:(h + 1) * HD], Ps[pv][:],
                            start=(pv == 0), stop=(pv == nkt - 1))
                        nc.tensor.matmul(
                            lps[:], ones_bf[:], Ps[pv][:],
                            start=(pv == 0), stop=(pv == nkt - 1))
                    if kt % 4 == 1 and fillers:
                        emit_proj_group(*fillers.pop(0))
                return (j, h, lps, yps)

            def emit_tail(j, h, lps, yps):
                """1/l partition-broadcast and y scaling into yT."""
                q0 = j * QB
                rinv = rp.tile([1, QB], F32, tag="rinv")
                nc.vector.reciprocal_approx_fast(out=rinv[:], in_=lps[:])
                rbc = rp.tile([128, QB], F32, tag="rbc")
                nc.gpsimd.partition_broadcast(rbc[:], rinv[:])
                nc.vector.tensor_tensor(
                    yT[:, h, q0:q0 + QB], yps[:], rbc[:], MULT)

            pend_tail = None
            proj_fifo = []   # (t, nb) proj groups awaiting emission
            for j in range(S // QB):
                for h in range(H_LOC):
                    state = emit_pair(j, h, proj_fifo)
                    if pend_tail is not None:
                        emit_tail(*pend_tail)
                        if pend_tail[1] == H_LOC - 1:
                            jj = pend_tail[0]
                            proj_fifo.extend(
                                (t, nb) for t in range(4 * jj, 4 * jj + 4)
                                for nb in range(D // 512))
                    pend_tail = state
            emit_tail(*pend_tail)
            jj = pend_tail[0]
            proj_fifo.extend(
                (t, nb) for t in range(4 * jj, 4 * jj + 4)
                for nb in range(D // 512))
            while proj_fifo:
                emit_proj_group(*proj_fifo.pop(0))
    nc.compile()
    return nc


def _prep_inputs(x, w_q_krope, w_kv_down, w_kv_up, w_proj, q_gain):
    """Build the 8 per-core input maps (host-side sharding, partition-major)."""

    def pmaj(a, p=128):
        """[K*p, ...] row-major -> [p, K, ...] partition-major contiguous."""
        k = a.shape[0] // p
        return np.ascontiguousarray(
            a.reshape((k, p) + a.shape[1:]).swapaxes(0, 1))

    inv_freq = ROPE_BASE ** (-np.arange(0, ROPE, 2, dtype=np.float32) / ROPE)
    t = np.arange(S, dtype=np.float32)
    freqs = np.outer(t, inv_freq)                      # (S, 32)
    cos4 = pmaj(np.ascontiguousarray(np.broadcast_to(
        np.cos(freqs)[:, None, :], (S, H_LOC, ROPE // 2))).astype(BF))
    sin4 = pmaj(np.ascontiguousarray(np.broadcast_to(
        np.sin(freqs)[:, None, :], (S, H_LOC, ROPE // 2))).astype(BF))

    kk = np.arange(128)[:, None, None]
    dd = np.arange(4)[None, :, None]
    qq = np.arange(QB)[None, None, :]
    masks = (kk + 128 * dd <= qq).astype(BF)           # [128, 4, QB]

    # x^T per batch, chunked partition-major: [X8_N, 128, KT_N, X8_CHUNK]
    xT_chunks = []
    for b in range(B):
        xT = np.ascontiguousarray(x[b].T).astype(BF)   # [D, S]
        ch = xT.reshape(KT_N, 128, X8_N, X8_CHUNK)
        xT_chunks.append(np.ascontiguousarray(ch.transpose(2, 1, 0, 3)))

    in_maps = []
    for c in range(N_CORES):
        b = c // H_LOC
        hg = c % H_LOC
        heads = [hg * H_LOC + i for i in range(H_LOC)]
        w_a = np.concatenate(
            [w_q_krope[:, h * HD:(h + 1) * HD] for h in heads]
            + [w_q_krope[:, D + h * ROPE:D + (h + 1) * ROPE] for h in heads]
            + [w_kv_down], axis=1).astype(BF)           # [D, 1280]
        w_up = np.concatenate(
            [w_kv_up[:, h * NOPE:(h + 1) * NOPE] for h in heads]
            + [w_kv_up[:, NOPE * H + h * HD:NOPE * H + (h + 1) * HD]
               for h in heads], axis=1).astype(BF)      # [LAT, 768]
        w_p = w_proj[hg * DLOC:(hg + 1) * DLOC, :].astype(BF)   # [512, D]
        g = q_gain[heads].astype(np.float32)
        g13 = np.concatenate([np.repeat(g, 2), np.ones(5, np.float32)])
        gain13 = np.ascontiguousarray(
            np.broadcast_to(g13[None, :], (128, 13))).astype(np.float32)
        in_maps.append({
            "xT8": xT_chunks[b],
            "w_a": pmaj(w_a),
            "w_up": pmaj(w_up),
            "w_p": pmaj(w_p),
            "cos4": cos4, "sin4": sin4, "masks": masks,
            "gain13": gain13,
        })
    return in_maps


def kernel(x, w_q_krope, w_kv_down, w_kv_up, w_proj, q_gain, **_unused):
    x = np.asarray(x, dtype=np.float32)
    w_q_krope = np.asarray(w_q_krope, dtype=np.float32)
    w_kv_down = np.asarray(w_kv_down, dtype=np.float32)
    w_kv_up = np.asarray(w_kv_up, dtype=np.float32)
    w_proj = np.asarray(w_proj, dtype=np.float32)
    q_gain = np.asarray(q_gain, dtype=np.float32)

    if "nc" not in _PROGRAM_CACHE:
        _PROGRAM_CACHE["nc"] = _build_program()
    nc = _PROGRAM_CACHE["nc"]

    in_maps = _prep_inputs(x, w_q_krope, w_kv_down, w_kv_up, w_proj, q_gain)
    res = run_bass_kernel_spmd(nc, in_maps, list(range(N_CORES)))

    out = np.zeros((B, S, D), dtype=np.float32)
    for c in range(N_CORES):
        out[c // H_LOC] += res.results[c]["out"]
    return out


# revision 23
# speedup vs baseline: 1.3959x; 1.0142x over previous
"""Multi-head latent attention (MLA) forward pass on 8 Trainium2 NeuronCores.

Sharding: 2 (batch) x 4 (head-group) grid. Core c handles batch b = c // 4
and heads 4*(c % 4) .. 4*(c % 4) + 3.

Design (measured 420 us vs 473 us for the PE-transpose baseline):
  - Zero PE transposes: all layout transposes (Q, [k_nope|k_rope], c_kv)
    run on the DMA xbar engine. The sync HWDGE queue carries ONLY xbar
    transposes; bulk DRAM loads go through the scalar HWDGE queue
    (partition-major DRAM layouts, x chunks prefetched one chunk ahead)
    and output stores through the gpsimd SWDGE queue, so transposes are
    never stuck behind megabyte transfers.
  - s-loop: GEMM-A (q|krope|kv_down, 1280 cols) + grouped rms-norm stats
    + rope per s-tile; kv_up GEMM software-pipelined with a 2-tile lag in
    its own single-buffered PSUM pool (6+2 banks) so neither GEMM ever
    WAR-waits on the previous tile's norms chain.
  - attention per (q-block j, head h), k-major scores S^T[k,q] so the exp
    output IS the PV moving operand (no P transpose). Per k-tile: score
    MM, exp on ACT, diagonal mask mult (DVE); PV and the ones^T @ P
    denominator matmuls trail the scores by 2 k-tiles so the ACT chain
    hides under PE streams. The 1/l reciprocal + gpsimd partition
    broadcast + y scaling are emitted one (j,h) pair late.
  - out projection is split into 16 PSUM-tile groups per q-block and
    emitted as PE filler inside the next block's pairs.
Host sums the 4 partials per batch element.
"""

import sys

for _p in ("/opt/trn_rl_repo",):
    if _p not in sys.path:
        sys.path.insert(0, _p)

import math
from contextlib import ExitStack

import ml_dtypes
import numpy as np

import concourse.bass as bass
import concourse.mybir as mybir
import concourse.tile as tile
from concourse import bacc
from concourse.bass_utils import run_bass_kernel_spmd

F32 = mybir.dt.float32
F32R = mybir.dt.float32r
BF16 = mybir.dt.bfloat16
BF = ml_dtypes.bfloat16

B, S, D = 2, 2048, 2048
H = 16
HD = 128           # head dim
ROPE = 64
NOPE = 64
LAT = 512
EPS = 1e-6
ROPE_BASE = 10000.0

H_LOC = 4          # heads per core
N_CORES = 8
DLOC = H_LOC * HD  # 512, per-core proj contraction size

ST_N = S // 128    # 16 s-tiles
KT_N = D // 128    # 16 k-tiles for GEMM-A
QB = 512           # attention q-block width

A_QW = H_LOC * HD             # 512  q columns in A
A_RW = H_LOC * ROPE           # 256  k_rope columns in A
A_W = A_QW + A_RW + LAT       # 1280 total A columns
KV_W = H_LOC * NOPE + H_LOC * HD   # 768 kv columns

X8_CHUNK = 256                # s-columns of x^T per streamed chunk
X8_N = S // X8_CHUNK          # 8 chunks
ST_PER_CHUNK = X8_CHUNK // 128  # 2

MULT = mybir.AluOpType.mult
ADD = mybir.AluOpType.add
SUB = mybir.AluOpType.subtract
EXPF = mybir.ActivationFunctionType.Exp
SQRTF = mybir.ActivationFunctionType.Sqrt
SQF = mybir.ActivationFunctionType.Square
AXX = mybir.AxisListType.X
AXXY = mybir.AxisListType.XY

_PROGRAM_CACHE = {}


def _build_program():
    nc = bacc.Bacc(None, target_bir_lowering=False, debug=True)

    # ---- DRAM I/O (all partition-major so SWDGE descriptors stay 1/part) --
    xT8 = nc.dram_tensor("xT8", [X8_N, 128, KT_N, X8_CHUNK], BF16,
                         kind="ExternalInput")
    w_a = nc.dram_tensor("w_a", [128, KT_N, A_W], BF16, kind="ExternalInput")
    w_up = nc.dram_tensor("w_up", [128, LAT // 128, KV_W], BF16,
                          kind="ExternalInput")
    w_p = nc.dram_tensor("w_p", [128, H_LOC, D], BF16, kind="ExternalInput")
    cos4 = nc.dram_tensor("cos4", [128, ST_N, H_LOC, ROPE // 2], BF16,
                          kind="ExternalInput")
    sin4 = nc.dram_tensor("sin4", [128, ST_N, H_LOC, ROPE // 2], BF16,
                          kind="ExternalInput")
    masks = nc.dram_tensor("masks", [128, 4, QB], BF16, kind="ExternalInput")
    gain13 = nc.dram_tensor("gain13", [128, 13], F32, kind="ExternalInput")
    out = nc.dram_tensor("out", [S, D], F32, kind="ExternalOutput")

    inv_sqrt_hd = 1.0 / math.sqrt(HD)

    with tile.TileContext(nc) as tc, ExitStack() as top:
        const = top.enter_context(tc.tile_pool(name="const", bufs=1))
        big = top.enter_context(tc.tile_pool(name="big", bufs=1))

        # --- persistent activations (head-dim-major) ---
        QT = big.tile([128, H_LOC, S], BF16)   # [d, h, q]
        KT = big.tile([128, H_LOC, S], BF16)   # [d, h, k] (0:64 nope, 64:128 rope)
        V = big.tile([128, ST_N, H_LOC * HD], BF16)  # [s%128, s//128, d_loc]
        yT = big.tile([128, H_LOC, S], BF16)   # [d, h, q]

        mask_sb = const.tile([128, 4, QB], BF16)
        gain_sb = const.tile([128, 13], F32)
        eps_sb = const.tile([128, 1], F32)
        ones_bf = const.tile([128, 1], BF16)
        wp_sb = const.tile([128, H_LOC, D], BF16)  # loaded late

        # ===================== phase S: s-tile loop =====================
        sphase = ExitStack()
        wpool = sphase.enter_context(tc.tile_pool(name="wpool", bufs=1))
        wa_sb = wpool.tile([128, KT_N, A_W], BF16)
        nc.scalar.dma_start(out=wa_sb[:, 0, :], in_=w_a[:, 0, :])

        x8p = sphase.enter_context(tc.tile_pool(name="x8p", bufs=2))
        xq0 = x8p.tile([128, KT_N, X8_CHUNK], BF16, tag="x8")
        nc.scalar.dma_start(out=xq0[:], in_=xT8[0])
        for kt in range(1, KT_N):
            nc.scalar.dma_start(out=wa_sb[:, kt, :], in_=w_a[:, kt, :])

        wup_sb = wpool.tile([128, LAT // 128, KV_W], BF16)
        nc.scalar.dma_start(out=wup_sb[:], in_=w_up[:])
        cos_sb = wpool.tile([128, ST_N, H_LOC, ROPE // 2], BF16)
        nc.scalar.dma_start(out=cos_sb[:], in_=cos4[:])
        sin_sb = wpool.tile([128, ST_N, H_LOC, ROPE // 2], BF16)
        nc.scalar.dma_start(out=sin_sb[:], in_=sin4[:])
        nc.scalar.dma_start(out=gain_sb[:], in_=gain13[:])
        nc.scalar.dma_start(out=mask_sb[:], in_=masks[:])
        nc.vector.memset(eps_sb[:], EPS)
        nc.vector.memset(ones_bf[:], 1.0)

        ckvT = big.tile([128, LAT // 128, S], BF16)  # [lat, lt, s]
        psA = sphase.enter_context(tc.tile_pool(name="psA", bufs=2, space="PSUM"))
        psKV = sphase.enter_context(tc.tile_pool(name="psKV", bufs=1, space="PSUM"))
        scr = sphase.enter_context(tc.tile_pool(name="scr", bufs=4))
        jnk = sphase.enter_context(tc.tile_pool(name="jnk", bufs=2))

        def rsqrt_act(dst, src, n):
            """dst = 1/sqrt(src/n + eps): ACT Sqrt then fast DVE reciprocal."""
            nc.scalar.activation(dst, src, SQRTF, scale=1.0 / n, bias=eps_sb[:])
            nc.vector.reciprocal_approx_fast(out=dst, in_=dst)

        xq1 = x8p.tile([128, KT_N, X8_CHUNK], BF16, tag="x8")
        nc.scalar.dma_start(out=xq1[:], in_=xT8[1])
        xq_next = {0: xq0, 1: xq1}

        def emit_gemm_a(ST):
            e, st2 = divmod(ST, ST_PER_CHUNK)
            if st2 == 0 and e + 1 < X8_N and (e + 1) not in xq_next:
                nxt = x8p.tile([128, KT_N, X8_CHUNK], BF16, tag="x8")
                nc.scalar.dma_start(out=nxt[:], in_=xT8[e + 1])
                xq_next[e + 1] = nxt
            xq = xq_next[e]
            aps = psA.tile([128, A_W], F32, tag="A")
            for kt in range(KT_N):
                lhs = xq[:, kt, st2 * 128:(st2 + 1) * 128]
                for c0, c1 in ((0, 512), (512, 1024), (1024, 1280)):
                    nc.tensor.matmul(
                        aps[:, c0:c1], lhs, wa_sb[:, kt, c0:c1],
                        start=(kt == 0), stop=(kt == KT_N - 1))
            return aps

        def emit_norms(ST, aps):
            s0 = ST * 128
            junk = jnk.tile([128, A_W], BF16, tag="junk")
            nc.scalar.activation(junk[:], aps[:], SQF)
            rs13 = scr.tile([128, 13], F32, tag="rs13")
            nc.vector.tensor_reduce(
                rs13[:, 0:12],
                junk[:, 0:768].rearrange("p (g c) -> p g c", c=64),
                AXX, ADD)
            nc.vector.tensor_reduce(
                rs13[:, 12:13],
                junk[:, 768:1280].rearrange("p (g c) -> p g c", c=64),
                AXXY, ADD)
            rsqrt_act(rs13[:, 0:12], rs13[:, 0:12], 64)
            rsqrt_act(rs13[:, 12:13], rs13[:, 12:13], LAT)
            nc.vector.tensor_tensor(rs13[:], rs13[:], gain_sb[:], MULT)

            # ---- apply norms ----
            nrmq = scr.tile([128, A_QW], BF16, tag="nrmq")
            nc.vector.tensor_tensor(
                nrmq[:].rearrange("p (g c) -> p g c", c=64),
                aps[:, 0:512].rearrange("p (g c) -> p g c", c=64),
                rs13[:, 0:8].to_broadcast([128, 8, 64]), MULT)
            kcomb = scr.tile([128, H_LOC, HD], BF16, tag="kcomb")
            nc.vector.tensor_tensor(
                kcomb[:, :, NOPE:HD],
                aps[:, 512:768].rearrange("p (h c) -> p h c", c=64),
                rs13[:, 8:12].to_broadcast([128, 4, 64]), MULT)
            cv = scr.tile([128, LAT], BF16, tag="cv")
            nc.vector.tensor_scalar(
                cv[:], aps[:, 768:1280], rs13[:, 12:13], None, MULT)

            # ---- rope (in place; temps carry all products first) ----
            RH = ROPE // 2
            cos_ap = cos_sb[:, ST]
            sin_ap = sin_sb[:, ST]
            nq = nrmq[:].rearrange("p (h t c) -> p h t c", t=2, c=64)

            def rope_inplace(x1, x2):
                t1 = scr.tile([128, H_LOC, RH], F32, tag="t1")
                t2 = scr.tile([128, H_LOC, RH], F32, tag="t2")
                t3 = scr.tile([128, H_LOC, RH], F32, tag="t3")
                t4 = scr.tile([128, H_LOC, RH], F32, tag="t4")
                nc.vector.tensor_tensor(t1[:], x1, cos_ap, MULT)
                nc.vector.tensor_tensor(t2[:], x2, sin_ap, MULT)
                nc.vector.tensor_tensor(t3[:], x2, cos_ap, MULT)
                nc.vector.tensor_tensor(t4[:], x1, sin_ap, MULT)
                nc.vector.tensor_tensor(x1, t1[:], t2[:], ADD)
                nc.vector.tensor_tensor(x2, t3[:], t4[:], SUB)

            rope_inplace(nq[:, :, 1, 0:RH], nq[:, :, 1, RH:ROPE])
            rope_inplace(kcomb[:, :, NOPE:NOPE + RH], kcomb[:, :, NOPE + RH:HD])

            # ---- DMA xbar transposes into head-dim-major tiles ----
            nc.sync.dma_start(
                out=QT[:, :, s0:s0 + 128], in_=nrmq[:], transpose=True)
            nc.sync.dma_start(
                out=ckvT[:, :, s0:s0 + 128], in_=cv[:], transpose=True)
            return kcomb

        def emit_kv_up(ST, kcomb):
            s0 = ST * 128
            kvps = psKV.tile([128, KV_W], F32, tag="KV")
            for lt in range(LAT // 128):
                lhs = ckvT[:, lt, s0:s0 + 128]
                for c0, c1 in ((0, 512), (512, 768)):
                    nc.tensor.matmul(
                        kvps[:, c0:c1], lhs, wup_sb[:, lt, c0:c1],
                        start=(lt == 0), stop=(lt == LAT // 128 - 1))
            junkk = jnk.tile([128, H_LOC * NOPE], BF16, tag="junkk")
            nc.scalar.activation(junkk[:], kvps[:, 0:256], SQF)
            rsk = scr.tile([128, 4], F32, tag="rsk")
            nc.vector.tensor_reduce(
                rsk[:], junkk[:].rearrange("p (g c) -> p g c", c=64),
                AXX, ADD)
            rsqrt_act(rsk[:], rsk[:], 64)
            nc.vector.tensor_tensor(
                kcomb[:, :, 0:NOPE],
                kvps[:, 0:256].rearrange("p (g c) -> p g c", c=64),
                rsk[:].to_broadcast([128, 4, 64]), MULT)
            nc.scalar.copy(V[:, ST, :], kvps[:, H_LOC * NOPE:KV_W])
            nc.sync.dma_start(
                out=KT[:, :, s0:s0 + 128], in_=kcomb[:], transpose=True)

        pend = []  # (ST, kcomb) with norms emitted, kv_up not yet (lag 2)
        for ST in range(ST_N):
            aps = emit_gemm_a(ST)
            if len(pend) >= 2:
                emit_kv_up(*pend.pop(0))
            kcomb = emit_norms(ST, aps)
            pend.append((ST, kcomb))
        for p in pend:
            emit_kv_up(*p)

        sphase.close()

        # late load for attention/proj phase
        nc.scalar.dma_start(out=wp_sb[:], in_=w_p[:])

        # =========== phase T: attention + out projection per q-block ========
        with (
            tc.tile_pool(name="pS", bufs=2, space="PSUM") as pS,
            tc.tile_pool(name="pY", bufs=2, space="PSUM") as pY,
            tc.tile_pool(name="pL", bufs=2, space="PSUM") as pL,
            tc.tile_pool(name="pO", bufs=2, space="PSUM") as pO,
            tc.tile_pool(name="pp", bufs=8) as pp,
            tc.tile_pool(name="rp", bufs=4) as rp,
            tc.tile_pool(name="po", bufs=4) as po,
        ):
            def emit_proj_group(t, nb):
                """One out-projection PSUM tile: 4 head matmuls + store."""
                s0 = t * 128
                ot = pO.tile([128, 512], F32, tag="O")
                for h in range(H_LOC):
                    nc.tensor.matmul(
                        ot[:], yT[:, h, s0:s0 + 128],
                        wp_sb[:, h, nb * 512:(nb + 1) * 512],
                        start=(h == 0), stop=(h == H_LOC - 1))
                osb = po.tile([128, 512], F32, tag="osb")
                nc.vector.tensor_copy(osb[:], ot[:])
                nc.gpsimd.dma_start(
                    out=out[s0:s0 + 128, nb * 512:(nb + 1) * 512],
                    in_=osb[:])

            def emit_pair(j, h, fillers):
                """Scores + PV + denominator for q-block j, head h. PV and
                the ones-matmul row-sum trail the scores by 2 k-tiles so
                the exp chain hides under matmul streams; `fillers` are
                pending proj groups emitted mid-pair as extra PE work."""
                q0 = j * QB
                nkt = 4 * j + 4
                lps = pL.tile([1, QB], F32, tag="L")
                yps = pY.tile([128, QB], F32, tag="Y")
                Ps = []
                for kt in range(nkt + 2):
                    if kt < nkt:
                        sps = pS.tile([128, QB], F32, tag="S")
                        nc.tensor.matmul(
                            sps[:], KT[:, h, kt * 128:(kt + 1) * 128],
                            QT[:, h, q0:q0 + QB], start=True, stop=True)
                        P = pp.tile([128, QB], BF16, tag="P")
                        nc.scalar.activation(
                            P[:], sps[:], EXPF, scale=inv_sqrt_hd)
                        d_idx = kt - 4 * j
                        if d_idx >= 0:
                            nc.vector.tensor_tensor(
                                P[:], P[:], mask_sb[:, d_idx, :], MULT)
                        Ps.append(P)
                    pv = kt - 2
                    if pv >= 0:
                        nc.tensor.matmul(
                            yps[:], V[:, pv, h * HD:(h + 1) * HD], Ps[pv][:],
                            start=(pv == 0), stop=(pv == nkt - 1))
                        nc.tensor.matmul(
                            lps[:], ones_bf[:], Ps[pv][:],
                            start=(pv == 0), stop=(pv == nkt - 1))
                    if kt % 4 == 1 and fillers:
                        emit_proj_group(*fillers.pop(0))
                return (j, h, lps, yps)

            def emit_tail(j, h, lps, yps):
                """1/l partition-broadcast and y scaling into yT."""
                q0 = j * QB
                rinv = rp.tile([1, QB], F32, tag="rinv")
                nc.vector.reciprocal_approx_fast(out=rinv[:], in_=lps[:])
                rbc = rp.tile([128, QB], F32, tag="rbc")
                nc.gpsimd.partition_broadcast(rbc[:], rinv[:])
                nc.vector.tensor_tensor(
                    yT[:, h, q0:q0 + QB], yps[:], rbc[:], MULT)

            pend_tail = None
            proj_fifo = []   # (t, nb) proj groups awaiting emission
            for j in range(S // QB):
                for h in range(H_LOC):
                    state = emit_pair(j, h, proj_fifo)
                    if pend_tail is not None:
                        emit_tail(*pend_tail)
                        if pend_tail[1] == H_LOC - 1:
                            jj = pend_tail[0]
                            proj_fifo.extend(
                                (t, nb) for t in range(4 * jj, 4 * jj + 4)
                                for nb in range(D // 512))
                    pend_tail = state
            emit_tail(*pend_tail)
            jj = pend_tail[0]
            proj_fifo.extend(
                (t, nb) for t in range(4 * jj, 4 * jj + 4)
                for nb in range(D // 512))
            while proj_fifo:
                emit_proj_group(*proj_fifo.pop(0))
    nc.compile()
    return nc


def _prep_inputs(x, w_q_krope, w_kv_down, w_kv_up, w_proj, q_gain):
    """Build the 8 per-core input maps (host-side sharding, partition-major)."""

    def pmaj(a, p=128):
        """[K*p, ...] row-major -> [p, K, ...] partition-major contiguous."""
        k = a.shape[0] // p
        return np.ascontiguousarray(
            a.reshape((k, p) + a.shape[1:]).swapaxes(0, 1))

    inv_freq = ROPE_BASE ** (-np.arange(0, ROPE, 2, dtype=np.float32) / ROPE)
    t = np.arange(S, dtype=np.float32)
    freqs = np.outer(t, inv_freq)                      # (S, 32)
    cos4 = pmaj(np.ascontiguousarray(np.broadcast_to(
        np.cos(freqs)[:, None, :], (S, H_LOC, ROPE // 2))).astype(BF))
    sin4 = pmaj(np.ascontiguousarray(np.broadcast_to(
        np.sin(freqs)[:, None, :], (S, H_LOC, ROPE // 2))).astype(BF))

    kk = np.arange(128)[:, None, None]
    dd = np.arange(4)[None, :, None]
    qq = np.arange(QB)[None, None, :]
    masks = (kk + 128 * dd <= qq).astype(BF)           # [128, 4, QB]

    # x^T per batch, chunked partition-major: [X8_N, 128, KT_N, X8_CHUNK]
    xT_chunks = []
    for b in range(B):
        xT = np.ascontiguousarray(x[b].T).astype(BF)   # [D, S]
        ch = xT.reshape(KT_N, 128, X8_N, X8_CHUNK)
        xT_chunks.append(np.ascontiguousarray(ch.transpose(2, 1, 0, 3)))

    in_maps = []
    for c in range(N_CORES):
        b = c // H_LOC
        hg = c % H_LOC
        heads = [hg * H_LOC + i for i in range(H_LOC)]
        w_a = np.concatenate(
            [w_q_krope[:, h * HD:(h + 1) * HD] for h in heads]
            + [w_q_krope[:, D + h * ROPE:D + (h + 1) * ROPE] for h in heads]
            + [w_kv_down], axis=1).astype(BF)           # [D, 1280]
        w_up = np.concatenate(
            [w_kv_up[:, h * NOPE:(h + 1) * NOPE] for h in heads]
            + [w_kv_up[:, NOPE * H + h * HD:NOPE * H + (h + 1) * HD]
               for h in heads], axis=1).astype(BF)      # [LAT, 768]
        w_p = w_proj[hg * DLOC:(hg + 1) * DLOC, :].astype(BF)   # [512, D]
        g = q_gain[heads].astype(np.float32)
        g13 = np.concatenate([np.repeat(g, 2), np.ones(5, np.float32)])
        gain13 = np.ascontiguousarray(
            np.broadcast_to(g13[None, :], (128, 13))).astype(np.float32)
        in_maps.append({
            "xT8": xT_chunks[b],
            "w_a": pmaj(w_a),
            "w_up": pmaj(w_up),
            "w_p": pmaj(w_p),
            "cos4": cos4, "sin4": sin4, "masks": masks,
            "gain13": gain13,
        })
    return in_maps


def kernel(x, w_q_krope, w_kv_down, w_kv_up, w_proj, q_gain, **_unused):
    x = np.asarray(x, dtype=np.float32)
    w_q_krope = np.asarray(w_q_krope, dtype=np.float32)
    w_kv_down = np.asarray(w_kv_down, dtype=np.float32)
    w_kv_up = np.asarray(w_kv_up, dtype=np.float32)
    w_proj = np.asarray(w_proj, dtype=np.float32)
    q_gain = np.asarray(q_gain, dtype=np.float32)

    if "nc" not in _PROGRAM_CACHE:
        _PROGRAM_CACHE["nc"] = _build_program()
    nc = _PROGRAM_CACHE["nc"]

    in_maps = _prep_inputs(x, w_q_krope, w_kv_down, w_kv_up, w_proj, q_gain)
    res = run_bass_kernel_spmd(nc, in_maps, list(range(N_CORES)))

    out = np.zeros((B, S, D), dtype=np.float32)
    for c in range(N_CORES):
        out[c // H_LOC] += res.results[c]["out"]
    return out
